# revision 47
# baseline (speedup 1.0000x reference)
"""Trainium2 Bass kernel for nn_Decoder_14894946583396 (dense_mlp).

Reference computation:
    sized = broadcast(representation[B,1,R] -> [B,S,R])   (ones @ rep)
    h     = relu(sized @ W1^T + b1)                       [B,S,HID]
    out   = h @ W2^T + b2                                 [B,S,OUT]

Every position s within batch b receives the identical input row, so
    row[b] = relu(rep[b] @ W1^T + b1) @ W2^T + b2         [B,OUT]
    out[b, s, :] = row[b]  for all s

Data-parallel across 8 NeuronCores: 4 batches per core, replicated
weights.  The per-core kernel keeps the DMA queues busy end to end:

  1. W1 then W2 stream on the SWDGE queue in chunks (W2 packed
     oc-major) while four tiny HWDGE inputs land in parallel: x^T+ones
     (bf16, SP ring); b1, selectors, and b2 (ACT ring).
  2. L1 is W1-stationary: per (rc,hc), lhsT = W1 block [128,128] (bf16,
     FWL), rhs = x^T block [128,4]; accumulates H^T [128h, 4m] directly
     -- no transposes.  rc-major so each W1 chunk is consumed as it
     lands; each hc accumulation group lives in its own PSUM bank
     (interleaving groups inside ONE bank corrupts accumulation).
  3. b1 enters as a K=1 ones-matmul; relu on ACT casting to bf16.
  4. L2 per oc half, started as its oc-major W2 chunk lands; Y [4,512]
     per oc in PSUM, groups sequential per bank.
  5. Per (batch, oc): a K=5 selector matmul replicates row[b] across
     all 128 partitions AND adds b2 (selector row 4 = ones, y row 4 =
     b2); DVE/ACT (batch parity) moves both halves of a batch to SBUF;
     a single 4 MiB DMA per batch (emitted right after its second
     half's copy) writes all S rows via a 0-stride broadcast source AP
     (32 KiB contiguous per partition on the DRAM side) -- no
     replication copies.

Single-sync-wait discipline: a BIR Matmult carries waits for BOTH
operands and any instruction may carry at most ONE semaphore wait;
HWDGE lane-reuse adds a non-elidable queue wait, so the HWDGE count is
kept at exactly 8 (xt, b1, sel, b2, 4 outputs).  Pre-observe edges
ride on instructions with a free wait slot (PE nop -> xt lane; bias
matmuls -> sel / b2 / w2-chunk-0 lanes).  b2 is DMA'd into row 4 of
the y tile (disjoint partitions from the DVE-written rows 0-3, so no
WAR).  Both PSUM->SBUF copies of a batch stay on one engine so its
output DMA needs a single wait.  A chain of 1-wait SP nops
pre-observes every final tick for the TileContext exit drain.
"""

import sys

import numpy as np

if "/opt/trn_rl_repo" not in sys.path:
    sys.path.insert(0, "/opt/trn_rl_repo")

B, S, R = 32, 1024, 1024
HID, OUT = 512, 1024
N_CORES = 8
BPC = B // N_CORES  # batches per core

RC = R // 128  # layer-1 contraction chunks
HC = HID // 128  # layer-2 contraction chunks
OC = OUT // 512  # 512-wide output column chunks
W1_CHUNKS = 2

N_COPIES = S // 128  # broadcast factor per output DMA (0-stride AP)

XT_W = RC * BPC + BPC  # x^T | ones
SEL_W = BPC * 128  # selector block width

_CACHED_NC = None


def _build_nc():
    import concourse.bass as bass
    import concourse.mybir as mybir
    from concourse.tile import TileContext, add_dep_helper

    f32 = mybir.dt.float32
    bf16 = mybir.dt.bfloat16
    relu = mybir.ActivationFunctionType.Relu
    fcopy = mybir.ActivationFunctionType.Copy
    nc = bass.Bass()

    xt = nc.dram_tensor("xt", [128, XT_W], bf16, kind="ExternalInput")
    aux = nc.dram_tensor("aux", [1, HID], bf16, kind="ExternalInput")
    selt = nc.dram_tensor("selt", [BPC + 1, SEL_W], f32, kind="ExternalInput")
    b2f = nc.dram_tensor("b2f", [1, OUT], f32, kind="ExternalInput")
    w1 = nc.dram_tensor("w1", [128, RC * HID], bf16, kind="ExternalInput")
    w2 = nc.dram_tensor("w2", [128, HC * OUT], bf16, kind="ExternalInput")
    # output stored bf16 on device (halves the dominant HBM write
    # stream); the host upcasts to f32 during the gather.  Rounding
    # adds <0.4% on top of the ~0.3% bf16-weight error, well under the
    # 2e-2 gate.
    out = nc.dram_tensor("out", [BPC, S, OUT], bf16, kind="ExternalOutput")

    ONE0 = RC * BPC  # ones row offset in xt

    with TileContext(nc) as tc:
        with (
            tc.tile_pool(name="const", bufs=1) as cpool,
            tc.tile_pool(name="psum_h", bufs=1, space="PSUM") as pp_h,
            tc.tile_pool(name="psum_y", bufs=1, space="PSUM") as pp_y,
            tc.tile_pool(name="psum_bc", bufs=2, space="PSUM") as pp_bc,
        ):
            # ---- input DMAs ------------------------------------------------
            xt_sb = cpool.tile([128, XT_W], bf16, tag="xt")
            dma_xt = nc.sync.dma_start(out=xt_sb[:, :], in_=xt[:, :])
            aux_sb = cpool.tile([1, HID], bf16, tag="aux")
            dma_aux = nc.scalar.dma_start(out=aux_sb[0:1, :], in_=aux[0:1, :])
            sel_sb = cpool.tile([BPC + 1, SEL_W], f32, tag="sel")
            dma_sel = nc.scalar.dma_start(out=sel_sb[:, :], in_=selt[:, :])
            # y_sb rows 0-3 = Y (written later by DVE); row 4 = b2 via DMA
            # (disjoint partitions, so no WAR between the two writers)
            y_sb = cpool.tile([BPC + 1, OUT], f32, tag="y")
            dma_b2 = nc.scalar.dma_start(out=y_sb[BPC : BPC + 1, :], in_=b2f[0:1, :])

            w1_sb = cpool.tile([128, RC * HID], bf16, tag="w1")
            w1_dmas = []
            w1_cols = RC * HID // W1_CHUNKS
            for c in range(W1_CHUNKS):
                d = nc.gpsimd.dma_start(
                    out=w1_sb[:, c * w1_cols : (c + 1) * w1_cols],
                    in_=w1[:, c * w1_cols : (c + 1) * w1_cols],
                )
                w1_dmas.append(d)
            # w2 packed oc-major, chunked on the same SWDGE queue
            w2_sb = cpool.tile([128, HC * OUT], bf16, tag="w2")
            w2_dmas = []
            w2_cols = HC * OUT // 2
            for c in range(2):
                d = nc.gpsimd.dma_start(
                    out=w2_sb[:, c * w2_cols : (c + 1) * w2_cols],
                    in_=w2[:, c * w2_cols : (c + 1) * w2_cols],
                )
                w2_dmas.append(d)

            # ---- L1: H^T[h, m] = W1 @ x (+b1), relu ------------------------
            # a PE nop pre-observes the xt lane so L1's matmuls carry only
            # their w1-chunk wait
            wn = nc.tensor.nop(nofuse=True)
            add_dep_helper(wn.ins, dma_xt.ins, sync=True, reason="observe xt")
            ph = []
            for hc in range(HC):
                ph_hc = pp_h.tile([128, BPC], f32, tag=f"h{hc}", name=f"ph{hc}")
                ph.append(ph_hc)
            # rc-major so each W1 chunk is consumed as it lands; one PSUM
            # bank per hc keeps each accumulation group sequential within
            # its bank.
            for rc in range(RC):
                for hc in range(HC):
                    nc.tensor.matmul(
                        ph[hc][:, :],
                        lhsT=w1_sb[:, rc * HID + hc * 128 : rc * HID + (hc + 1) * 128],
                        rhs=xt_sb[:, rc * BPC : (rc + 1) * BPC],
                        start=(rc == 0),
                        stop=False,
                        skip_group_check=True,
                    )
            # b1 as a K=1 ones-matmul: ph[h, m] += b1[h] * 1; later ones
            # have free wait slots to pre-observe upcoming DMA lanes
            for hc in range(HC):
                bmm = nc.tensor.matmul(
                    ph[hc][:, :],
                    lhsT=aux_sb[0:1, hc * 128 : (hc + 1) * 128],
                    rhs=xt_sb[0:1, ONE0 : ONE0 + BPC],
                    start=False,
                    stop=True,
                    skip_group_check=True,
                )
                if hc == 1:
                    add_dep_helper(bmm.ins, dma_sel.ins, sync=True, reason="obs sel")
                if hc == 2:
                    add_dep_helper(bmm.ins, dma_b2.ins, sync=True, reason="obs b2")
                if hc == 3:
                    add_dep_helper(
                        bmm.ins, w2_dmas[0].ins, sync=True, reason="obs w2c0"
                    )
            ht_sb = cpool.tile([128, HC * BPC], bf16, tag="ht")
            for hc in range(HC):
                nc.scalar.activation(
                    ht_sb[:, hc * BPC : (hc + 1) * BPC],
                    ph[hc][:, :],
                    relu,
                )

            # ---- L2 + broadcast + store, per oc half -----------------------
            py = []
            for oc in range(OC):
                py_oc = pp_y.tile([BPC, 512], f32, tag=f"y{oc}", name=f"py{oc}")
                py.append(py_oc)

            out_dmas = []
            last_act = None
            last_dve = None
            yts = []
            for b in range(BPC):
                yt_b = cpool.tile([128, OUT], bf16, tag=f"yt{b}", name=f"yt{b}")
                yts.append(yt_b)
            for oc in range(OC):
                for hc in range(HC):
                    nc.tensor.matmul(
                        py[oc][:, :],
                        lhsT=ht_sb[:, hc * BPC : (hc + 1) * BPC],
                        rhs=w2_sb[
                            :, oc * HC * 512 + hc * 512 : oc * HC * 512 + (hc + 1) * 512
                        ],
                        start=(hc == 0),
                        stop=(hc == HC - 1),
                        skip_group_check=True,
                    )
                last_dve = nc.vector.tensor_copy(
                    y_sb[0:BPC, oc * 512 : (oc + 1) * 512], py[oc][:, :]
                )
                for b in range(BPC):
                    pb = pp_bc.tile([128, 512], f32, tag="bc", name=f"pb{b}_{oc}")
                    mm = nc.tensor.matmul(
                        pb[:, :],
                        lhsT=sel_sb[0 : BPC + 1, b * 128 : (b + 1) * 128],
                        rhs=y_sb[0 : BPC + 1, oc * 512 : (oc + 1) * 512],
                        start=True,
                        stop=True,
                    )
                    last_mm = mm
                    dst = yts[b][:, oc * 512 : (oc + 1) * 512]
                    if b % 2 == 0:
                        last_dve = nc.vector.tensor_copy(dst, pb[:, :])
                    else:
                        last_act = nc.scalar.activation(dst, pb[:, :], fcopy)
                    if oc == OC - 1:
                        # one DMA writes all S rows of batch b via a
                        # 0-stride broadcast source AP
                        d = nc.sync.dma_start(
                            out=out[b, :, :].rearrange(
                                "(p c) o -> p c o", c=N_COPIES
                            ),
                            in_=yts[b][:, :]
                            .rearrange("p (c o) -> p c o", c=1)
                            .to_broadcast((128, N_COPIES, OUT)),
                        )
                        out_dmas.append(d)

            # single-sync-wait discipline for the TileContext exit drain:
            # chain SP nops, one dependency each, so SP's vector clock
            # observes every DMA lane / engine tick before the drain.
            tail = (
                out_dmas
                + w1_dmas
                + w2_dmas
                + [dma_xt, dma_sel, dma_aux, dma_b2, last_mm, last_act, last_dve]
            )
            tail = [t for t in tail if t is not None]
            for d in tail:
                tn = nc.sync.nop(nofuse=True)
                add_dep_helper(
                    tn.ins, d.ins, sync=True, reason="observe final ticks pre-drain"
                )

    return nc


def _get_nc():
    global _CACHED_NC
    if _CACHED_NC is None:
        _CACHED_NC = _build_nc()
    return _CACHED_NC


def _prep_in_maps(representation, W1, b1, W2, b2):
    import ml_dtypes

    bf16 = ml_dtypes.bfloat16

    rep = np.asarray(representation, dtype=np.float32).reshape(B, R)
    w1 = np.asarray(W1, dtype=np.float32)
    w2 = np.asarray(W2, dtype=np.float32)
    b1 = np.asarray(b1, dtype=np.float32)
    b2 = np.asarray(b2, dtype=np.float32)

    # w1p[p, rc*HID + hc*128 + j] = W1[hc*128+j, rc*128+p]
    w1p = np.ascontiguousarray(
        w1.reshape(HC, 128, RC, 128).transpose(3, 2, 0, 1).reshape(128, RC * HID)
    ).astype(bf16)
    # w2p[p, oc*HC*512 + hc*512 + o] = W2[oc*512+o, hc*128+p]  (oc-major)
    w2p = np.ascontiguousarray(
        w2.reshape(OC, 512, HC, 128).transpose(3, 0, 2, 1).reshape(128, HC * OUT)
    ).astype(bf16)
    # aux row: b1 (bf16)
    auxp = b1.reshape(1, HID).astype(bf16)
    # selectors: sel[k, b*128 + i] = (k == b); row BPC = ones (b2 gate)
    selp = np.zeros((BPC + 1, SEL_W), dtype=np.float32)
    for b in range(BPC):
        selp[b, b * 128 : (b + 1) * 128] = 1.0
    selp[BPC, :] = 1.0
    b2p = b2.reshape(1, OUT).copy()

    in_maps = []
    for c in range(N_CORES):
        xtc = rep[c * BPC : (c + 1) * BPC].T  # [R, BPC]
        # xt[p, rc*BPC + m] = rep[m, rc*128+p] | ones row | b1 row
        xtp = np.zeros((128, XT_W), dtype=np.float32)
        xtp[:, 0 : RC * BPC] = (
            xtc.reshape(RC, 128, BPC).transpose(1, 0, 2).reshape(128, RC * BPC)
        )
        xtp[0, RC * BPC : RC * BPC + BPC] = 1.0
        in_maps.append(
            {
                "xt": xtp.astype(bf16),
                "aux": auxp,
                "selt": selp,
                "b2f": b2p,
                "w1": w1p,
                "w2": w2p,
            }
        )
    return in_maps


def run_sharded(representation, W1, b1, W2, b2, **run_kwargs):
    """Compile+run on 8 cores; returns (full_output, BassKernelResults)."""
    from concourse.bass_utils import run_bass_kernel_spmd

    nc = _get_nc()
    in_maps = _prep_in_maps(representation, W1, b1, W2, b2)
    res = run_bass_kernel_spmd(nc, in_maps, core_ids=list(range(N_CORES)), **run_kwargs)
    full = np.concatenate(
        [np.asarray(r["out"]).astype(np.float32) for r in res.results], axis=0
    )
    return full, res


def kernel(representation, size_matrix=None, W1=None, b1=None, W2=None, b2=None):
    # size_matrix only contributes its shape in the reference (ones_like);
    # its values are unused.
    full, _ = run_sharded(representation, W1, b1, W2, b2)
    return full
